# revision 13
# baseline (speedup 1.0000x reference)
"""Self-contained Trainium2 Bass kernel for 16-head cross-attention MHA.

Problem: B=2, SQ=SK=2048, D=1024, H=16, key_size=64 (fp32 in/out).

Sharding (8 cores): data-parallel over batch (2) x tensor-parallel over
head groups (4 heads per core). Each core computes its 4 heads'
Q/K/V projections (column slices of wq/wk/wv), attention, and a partial
output projection (row slice of wo). Host sums the 4 partial outputs per
batch and adds the (bv @ wo + bo) correction (probs sum to 1, so bv
contributes exactly bv @ wo; bk cancels in softmax).

Device pipeline per core (bf16 matmuls, fp32 PSUM accumulation — fp8
variants measured 2-5% output error, over the gate). The kernel is
ScalarE-paced: exp over the 4 x 2048 x 2048 score matrix is ~142us of
ACT time; everything else is scheduled to hide under it:

  1. Prefix: host pre-arranges every tensor into its SBUF layout so
     each DMA is 16-32KB-contiguous per partition (~3.5x the DMA
     throughput of the 512B-descriptor rearranged forms). xeT then xdT
     kt-pair chunks alternate the sync/scalar queues (weights ride
     gpsimd; wo is gated behind the K eviction). ~30 junk matmuls keep
     the PE out of its low p-states while the DMAs stream; K-nt0 and
     Q-nt0 (m-half 0 only) chains pipeline behind the chunk arrivals.
  2. Phases (head-pair, m-half): scores^T tiles ([key_pos, query]
     layout) exp'd by ScalarE on [128,1024] PSUM tiles (scale=1/8
     fused) into bf16 SBUF, 38-buffer pool — deep enough that phase
     i+1's exps never stall on phase i's shifted ctx (the 34-buffer
     version lost ~8us/phase to pool lockstep).
  3. ctx for phase i is PHASE-SHIFTED into phase i+1 (V' ones column
     accumulates the softmax denominator Z); the last phase runs prev
     ctx at 2x front / own ctx at 2x back. Phase-0 dribbles: V' pairs
     (jts 0-7), then single-PSUM-bank sequential chains for Q-nt0
     m-half1 + K-nt1 (jts 8-11) and Q-nt1 (jts 12-15).
  4. Normalization reads ctx straight from PSUM (no staging copy):
     Z row partition-broadcast via GpSimd, DVE reciprocal + multiply
     into ctx^T bf16.
  5. Tail: out-proj for the already-normed m-half (mts 0-7, ACT-only
     evictions) runs on PE while DVE/GpSimd norm the last phase's ctx;
     then mts 8-15 with ACT/DVE split evictions. Output in bf16 (host
     sums partials in fp32), halving the closing DMA drain.
"""

import os
import sys

for _p in ("/opt/trn_rl_repo", "/root/.axon_site/_ro/trn_rl_repo"):
    if os.path.isdir(_p) and _p not in sys.path:
        sys.path.insert(0, _p)

import numpy as np
import ml_dtypes

BF16 = ml_dtypes.bfloat16

B = 2
S = 2048          # SQ == SK
D = 1024
H = 16
KEY = 64
HPC = 4           # heads per core
NPC = HPC * KEY   # 256 per-core slice of D
KT = D // 128     # 8 contraction tiles for projections
TP = KT // 2      # 4 kt-pair DMA chunks
NT = NPC // 128   # 2 head-pair tiles
MC = S // 512     # 4 m-chunks of 512
JT = S // 128     # 16 key tiles

_NC = None
LAST_RESULTS = None  # BassKernelResults of the most recent run (for test.py)


def _build_nc():
    import concourse.tile as tile
    from concourse import bacc, mybir

    FP32 = mybir.dt.float32
    BF = mybir.dt.bfloat16
    AF = mybir.ActivationFunctionType

    nc = bacc.Bacc("TRN2", target_bir_lowering=False, debug=False, num_devices=8)

    xdT_d = nc.dram_tensor("xdT", [128 * KT * S], BF, kind="ExternalInput").ap()
    xeT_d = nc.dram_tensor("xeT", [128 * KT * S], BF, kind="ExternalInput").ap()
    wq_d = nc.dram_tensor("wq", [128 * KT * NPC], BF, kind="ExternalInput").ap()
    wk_d = nc.dram_tensor("wk", [128 * KT * NPC], BF, kind="ExternalInput").ap()
    wv_d = nc.dram_tensor("wv", [128 * KT * NPC], BF, kind="ExternalInput").ap()
    wo_d = nc.dram_tensor("wo", [128 * NT * D], BF, kind="ExternalInput").ap()
    bq_d = nc.dram_tensor("bq", [NT, 128, 1], FP32, kind="ExternalInput").ap()
    o_d = nc.dram_tensor("o", [S, D], BF, kind="ExternalOutput").ap()

    with tile.TileContext(nc) as tc:
        with (
            tc.tile_pool(name="consts", bufs=1) as consts,
            tc.tile_pool(name="acts", bufs=1) as acts,
            tc.tile_pool(name="zp", bufs=2) as zp,
            tc.tile_pool(name="zbp", bufs=2) as zbp,
            tc.tile_pool(name="osb", bufs=3) as osb,
        ):
            # Host pre-arranges every tensor into its SBUF layout, so each
            # DMA is one 16-32KB contiguous descriptor per partition (the
            # rearranged 512B-descriptor forms measured ~3x lower DMA
            # throughput). xeT/xdT halves split across the sync/scalar
            # queues so the two queue rails stream in parallel.
            xeT_r = xeT_d.rearrange("(p t m) -> p t m", p=128, t=KT)
            xdT_r = xdT_d.rearrange("(p t m) -> p t m", p=128, t=KT)

            wk_sb = consts.tile([128, KT, NPC], BF, tag="wk")
            nc.gpsimd.dma_start(wk_sb[:], wk_d.rearrange("(p t n) -> p t n", p=128, t=KT))
            wq_sb = consts.tile([128, KT, NPC], BF, tag="wq")
            nc.gpsimd.dma_start(wq_sb[:], wq_d.rearrange("(p t n) -> p t n", p=128, t=KT))
            wv_sb = consts.tile([128, KT, NPC], BF, tag="wv")
            nc.gpsimd.dma_start(wv_sb[:], wv_d.rearrange("(p t n) -> p t n", p=128, t=KT))
            wo_sb = consts.tile([128, NT, D], BF, tag="wo")

            # xeT first (K0 gates on it), then xdT m-half0 (Q0 prefix), then
            # xdT m-half1 (phase-0 dribbles). kt-pair chunks alternate the
            # sync/scalar queues so K0/Q0 chains pipeline behind arrivals.
            xeT_sb = consts.tile([128, KT, S], BF, tag="xeT")
            xdT_sb = consts.tile([128, KT, S], BF, tag="xdT")
            qs = [nc.sync, nc.scalar]
            for tp in range(TP):
                qs[tp % 2].dma_start(xeT_sb[:, 2 * tp:2 * tp + 2, :],
                                     xeT_r[:, 2 * tp:2 * tp + 2, :])
            for mh2 in range(2):
                msl = slice(mh2 * 1024, (mh2 + 1) * 1024)
                for tp in range(TP):
                    qs[tp % 2].dma_start(xdT_sb[:, 2 * tp:2 * tp + 2, msl],
                                         xdT_r[:, 2 * tp:2 * tp + 2, msl])
            bq_sb = consts.tile([128, NT, 1], FP32, tag="bq")
            nc.scalar.dma_start(bq_sb[:], bq_d.rearrange("t p o -> p t o"))

            # ---- activations kept resident ----
            QT_sb = acts.tile([128, NT, S], BF, tag="QT")    # [head_dim, m]
            KT_sb = acts.tile([128, NT, S], BF, tag="KT")    # [head_dim, j]
            v_sb = acts.tile([128, JT, HPC, KEY + 1], BF, tag="v")  # V' + ones col
            ctxT_sb = acts.tile([128, NT, S], BF, tag="ctxT")

            nc.vector.memset(v_sb[:, :, :, KEY:KEY + 1], 1.0)

            # ================= single PSUM pool =================
            # "ss": 2x[128,1024] (4 banks) scores / out-proj
            # "cc": 4x[128,512] (4 banks) proj chains, V chains, ctx chains
            with (
                tc.tile_pool(name="expp", bufs=38) as expp,
                tc.tile_pool(name="ps", bufs=2, space="PSUM") as ps,
                tc.tile_pool(name="cp", bufs=4, space="PSUM") as cp,
            ):
                def proj_pass(w_sb, x_sb, nt, chains, mcs=range(MC)):
                    for kt in range(KT):
                        for i, mc in enumerate(mcs):
                            nc.tensor.matmul(
                                chains[i][:],
                                w_sb[:, kt, nt * 128:(nt + 1) * 128],
                                x_sb[:, kt, mc * 512:(mc + 1) * 512],
                                start=(kt == 0),
                                stop=(kt == KT - 1),
                            )

                def evict_proj(chains, dst, nt, bias, mcs=range(MC)):
                    for i, mc in enumerate(mcs):
                        out_ap = dst[:, nt, mc * 512:(mc + 1) * 512]
                        if bias is not None:
                            nc.vector.tensor_scalar_add(out_ap, chains[i][:],
                                                        bias[:, nt, :])
                        else:
                            nc.vector.tensor_copy(out_ap, chains[i][:])

                # ---- PE warm-up: ~30 junk matmuls on a zero tile keep the
                # Tensor engine out of its low p-states while the prefix
                # DMAs stream; K0 then runs at full clock.
                warm_sb = consts.tile([128, 512], BF, tag="warm")
                nc.vector.memset(warm_sb[:], 0.0)
                wp = cp.tile([128, 512], FP32, tag="cc", name="warm")
                for _ in range(30):
                    nc.tensor.matmul(wp[:], warm_sb[:, 0:128], warm_sb[:],
                                     start=True, stop=True)

                # ---- prefix: K-nt0, then Q-nt0 for m-half 0 only ----
                k0 = [cp.tile([128, 512], FP32, tag="cc", name=f"k0_{i}")
                      for i in range(4)]
                proj_pass(wk_sb, xeT_sb, 0, k0)
                evict_proj(k0, KT_sb, 0, None)
                # wo (tail-only, 512KB) rides the gpsimd queue gated on a
                # dummy read of the K0 eviction so it cannot steal DMA
                # bandwidth from the critical xeT/xdT prefix loads.
                gate = consts.tile([1, 8], BF, tag="gate")
                nc.gpsimd.tensor_copy(gate[:], KT_sb[0:1, 0, 0:8])
                nc.gpsimd.dma_start(wo_sb[:],
                                    wo_d.rearrange("(p t n) -> p t n", p=128, t=NT))
                q0 = [cp.tile([128, 512], FP32, tag="cc", name=f"q0_{i}")
                      for i in range(2)]
                proj_pass(wq_sb, xdT_sb, 0, q0, mcs=(0, 1))
                evict_proj(q0, QT_sb, 0, bq_sb, mcs=(0, 1))

                def emit_outproj(mts, split):
                    for mt in mts:
                        ot = osb.tile([128, D], BF, tag="ot")
                        po = ps.tile([128, 1024], FP32, tag="ss",
                                     name=f"po_{mt}")
                        for dt in range(NT):
                            for ec in range(2):
                                nc.tensor.matmul(
                                    po[:, ec * 512:(ec + 1) * 512],
                                    ctxT_sb[:, dt, mt * 128:(mt + 1) * 128],
                                    wo_sb[:, dt, ec * 512:(ec + 1) * 512],
                                    start=(dt == 0),
                                    stop=(dt == NT - 1),
                                )
                        if split:
                            nc.scalar.copy(ot[:, 0:512], po[:, 0:512])
                            nc.vector.tensor_copy(ot[:, 512:1024],
                                                  po[:, 512:1024])
                        else:
                            nc.scalar.copy(ot[:], po[:])
                        if mt % 2 == 0:
                            nc.sync.dma_start(
                                o_d[mt * 128:(mt + 1) * 128, :], ot[:])
                        else:
                            nc.gpsimd.dma_start(
                                o_d[mt * 128:(mt + 1) * 128, :], ot[:])

                # ---- phases: scores(si) + shifted ctx(si-1) + dribbles ----
                order = [(0, 0), (0, 1), (1, 0), (1, 1)]
                rows = [0, KEY]
                prev = None  # (hp, mh, exp_tiles)
                drib = {}    # state for phase-0 dribbles

                def emit_ctx_half(hp, mh, jt, exp_row, ccs, hh):
                    h = hp * 2 + hh
                    for q in range(2):
                        nc.tensor.matmul(
                            ccs[hh * 2 + q][0:KEY + 1, :],
                            v_sb[:, jt, h, :],
                            exp_row[hh][:, q * 512:(q + 1) * 512],
                            start=(jt == 0),
                            stop=(jt == JT - 1),
                        )

                def emit_ctx_step(hp, mh, jt, exp_row, ccs):
                    for hh in range(2):
                        h = hp * 2 + hh
                        for q in range(2):
                            nc.tensor.matmul(
                                ccs[hh * 2 + q][0:KEY + 1, :],
                                v_sb[:, jt, h, :],
                                exp_row[hh][:, q * 512:(q + 1) * 512],
                                start=(jt == 0),
                                stop=(jt == JT - 1),
                            )

                def emit_norm(hp, mh, ccs, qs=(0, 1)):
                    m0 = mh * 1024
                    for q in qs:
                        for hh in range(2):
                            row = rows[hh]
                            c = ccs[hh * 2 + q]
                            zraw = zp.tile([1, 512], FP32, tag="z")
                            nc.vector.tensor_copy(zraw[:], c[KEY:KEY + 1, :])
                            zb = zbp.tile([KEY, 512], FP32, tag="zb")
                            nc.gpsimd.partition_broadcast(zb[:], zraw[:])
                            zbr = zbp.tile([KEY, 512], FP32, tag="zbr")
                            nc.vector.reciprocal_approx_fast(zbr[:], zb[:])
                            nc.vector.tensor_mul(
                                ctxT_sb[row:row + KEY, hp,
                                        m0 + q * 512:m0 + (q + 1) * 512],
                                c[0:KEY, :],
                                zbr[:],
                            )

                for si, (hp, mh) in enumerate(order):
                    m0 = mh * 1024
                    last = si == len(order) - 1
                    cur_cc = None
                    prev_cc = None
                    if prev is not None:
                        prev_cc = [cp.tile([128, 512], FP32, tag="cc",
                                           name=f"cc_{si}_{i}")
                                   for i in range(4)]
                    cur_exps = []
                    for jt in range(JT):
                        exp_row = []
                        for hh in range(2):
                            row = rows[hh]
                            ss = ps.tile([128, 1024], FP32, tag="ss")
                            for q in range(2):
                                nc.tensor.matmul(
                                    ss[:, q * 512:(q + 1) * 512],
                                    KT_sb[row:row + KEY, hp,
                                          jt * 128:(jt + 1) * 128],
                                    QT_sb[row:row + KEY, hp,
                                          m0 + q * 512:m0 + (q + 1) * 512],
                                    start=True, stop=True,
                                )
                            et = expp.tile([128, 1024], BF, tag="exp")
                            nc.scalar.activation(et[:], ss[:], AF.Exp,
                                                 scale=0.125)
                            exp_row.append(et)
                            # half the shifted-ctx matmuls between the two
                            # score pairs: their streams cover the scores
                            # LDWEIGHTS latency.
                            if prev is not None and not last:
                                emit_ctx_half(prev[0], prev[1], jt,
                                              prev[2][jt], prev_cc, hh)
                        cur_exps.append(exp_row)
                        if last:
                            # front half: finish prev phase's ctx at 2x rate;
                            # back half: this phase's own ctx at 2x rate.
                            if jt < 8:
                                for j2 in (jt * 2, jt * 2 + 1):
                                    emit_ctx_step(prev[0], prev[1], j2,
                                                  prev[2][j2], prev_cc)
                                if jt == 7:
                                    emit_norm(prev[0], prev[1], prev_cc)
                            else:
                                if jt == 8:
                                    cur_cc = [cp.tile([128, 512], FP32,
                                                      tag="cc",
                                                      name=f"cc_last_{i}")
                                              for i in range(4)]
                                for j2 in ((jt - 8) * 2, (jt - 8) * 2 + 1):
                                    emit_ctx_step(hp, mh, j2, cur_exps[j2],
                                                  cur_cc)
                        if si == 0:
                            # jts 0-3: Q-nt0 m-half1 (needed by phase 1);
                            # jts 0-9: V tiles (phase 1's shifted ctx);
                            # jts 10-13: K-nt1; jts 12-15: Q-nt1 (first
                            # needed by phase 2 / phase 3 scores).
                            if 8 <= jt < 12:
                                if jt == 8:
                                    drib["q0b"] = [
                                        cp.tile([128, 512], FP32, tag="cc",
                                                name=f"q0b_{i}")
                                        for i in range(2)]
                                for kk in range(2):
                                    kt = (jt - 8) * 2 + kk
                                    for i, mc in enumerate((2, 3)):
                                        nc.tensor.matmul(
                                            drib["q0b"][i][:],
                                            wq_sb[:, kt, 0:128],
                                            xdT_sb[:, kt,
                                                   mc * 512:(mc + 1) * 512],
                                            start=(kt == 0), stop=(kt == 7),
                                        )
                                if jt == 11:
                                    evict_proj(drib["q0b"], QT_sb, 0, bq_sb,
                                               mcs=(2, 3))
                            # V tile pairs at jts 0-7
                            if jt < 8:
                                for vtile in (2 * jt, 2 * jt + 1):
                                    pv = cp.tile([128, 512], FP32, tag="cc",
                                                 name=f"pv_{vtile}")
                                    for kt in range(KT):
                                        nc.tensor.matmul(
                                            pv[:, 0:NPC],
                                            xeT_sb[:, kt,
                                                   vtile * 128:
                                                   (vtile + 1) * 128],
                                            wv_sb[:, kt, :],
                                            start=(kt == 0),
                                            stop=(kt == KT - 1),
                                        )
                                    nc.vector.tensor_copy(
                                        v_sb[:, vtile, :, 0:KEY],
                                        pv[:, 0:NPC].rearrange(
                                            "p (h k) -> p h k", h=HPC),
                                    )
                            # K-nt1 / Q-nt1: one single-bank chain per jt
                            # (sequential m-chunks keep PSUM to 1 bank each).
                            if 8 <= jt < 12:
                                mc = jt - 8
                                kc = cp.tile([128, 512], FP32, tag="cc",
                                             name=f"k1_{mc}")
                                for kt in range(KT):
                                    nc.tensor.matmul(
                                        kc[:],
                                        wk_sb[:, kt, 128:256],
                                        xeT_sb[:, kt,
                                               mc * 512:(mc + 1) * 512],
                                        start=(kt == 0), stop=(kt == 7),
                                    )
                                evict_proj([kc], KT_sb, 1, None, mcs=(mc,))
                            if 12 <= jt < 16:
                                mc = jt - 12
                                qc = cp.tile([128, 512], FP32, tag="cc",
                                             name=f"q1_{mc}")
                                for kt in range(KT):
                                    nc.tensor.matmul(
                                        qc[:],
                                        wq_sb[:, kt, 128:256],
                                        xdT_sb[:, kt,
                                               mc * 512:(mc + 1) * 512],
                                        start=(kt == 0), stop=(kt == 7),
                                    )
                                evict_proj([qc], QT_sb, 1, bq_sb, mcs=(mc,))
                    if last:
                        # out-proj for the m-half whose ctx is already
                        # normed (mh0, mts 0-7) runs on PE while DVE/GpSimd
                        # norm this phase's ctx quarter by quarter;
                        # evictions stay on ACT so DVE is free for the norm.
                        emit_outproj(range(0, 8), split=False)
                        emit_norm(hp, mh, cur_cc, qs=(0,))
                        emit_outproj(range(8, 12), split=False)
                        emit_norm(hp, mh, cur_cc, qs=(1,))
                        emit_outproj(range(12, 16), split=True)
                    elif prev is not None:
                        emit_norm(prev[0], prev[1], prev_cc)
                    prev = (hp, mh, cur_exps)

    nc.compile()
    return nc


def _get_nc():
    global _NC
    if _NC is None:
        _NC = _build_nc()
    return _NC


def _maybe_register_ntff_hook():
    """Optional: register the axon NTFF profile hook so BASS_TRACE=1 yields
    HW exec times. No-op if unavailable (e.g. the grading environment)."""
    if "antenv.axon_hooks" in sys.modules:
        return
    try:
        import types

        if "/root/.axon_site" not in sys.path and os.path.isdir("/root/.axon_site"):
            sys.path.append("/root/.axon_site")
        from trn_agent_boot.trn_boot import _ntff_profile_via_ctypes

        hook = _ntff_profile_via_ctypes("/opt/axon/libaxon_pjrt.so")
        mod = types.ModuleType("antenv.axon_hooks")
        mod.get_axon_ntff_profile_hook = lambda: hook
        mod.set_axon_ntff_profile_hook = lambda h: None
        sys.modules["antenv.axon_hooks"] = mod
    except Exception:
        pass


def kernel(decoder_output, encoder_output, wq, bq, wk, bk, wv, bv, wo, bo):
    from concourse.bass_utils import run_bass_kernel_spmd

    global LAST_RESULTS

    decoder_output = np.asarray(decoder_output, dtype=np.float32)
    encoder_output = np.asarray(encoder_output, dtype=np.float32)
    wq = np.asarray(wq, dtype=np.float32)
    wk = np.asarray(wk, dtype=np.float32)
    wv = np.asarray(wv, dtype=np.float32)
    wo = np.asarray(wo, dtype=np.float32)
    bq = np.asarray(bq, dtype=np.float32)
    bv = np.asarray(bv, dtype=np.float32)
    bo = np.asarray(bo, dtype=np.float32)
    # bk is softmax-invariant (adds a per-query constant to every logit).

    if os.environ.get("BASS_TRACE"):
        _maybe_register_ntff_hook()

    nc = _get_nc()

    def arr_x(x):  # [S, D] -> flat [p, kt, m] per-partition-contiguous
        t = x.T.reshape(KT, 128, S).transpose(1, 0, 2)  # [p, kt, m]
        return np.ascontiguousarray(t).astype(BF16).reshape(-1)

    def arr_w(w):  # [D, n] -> flat [p, kt, n]
        n = w.shape[1]
        t = w.reshape(KT, 128, n).transpose(1, 0, 2)
        return np.ascontiguousarray(t).astype(BF16).reshape(-1)

    xT = {}
    for b in range(B):
        xT[("d", b)] = arr_x(decoder_output[b])
        xT[("e", b)] = arr_x(encoder_output[b])

    in_maps = []
    for c in range(8):
        b, hg = c // 4, c % 4
        sl = slice(hg * NPC, (hg + 1) * NPC)
        wo_sl = np.ascontiguousarray(wo[sl, :])  # [NPC, D]
        wo_t = wo_sl.reshape(NT, 128, D).transpose(1, 0, 2)
        in_maps.append({
            "xdT": xT[("d", b)],
            "xeT": xT[("e", b)],
            "wq": arr_w(wq[:, sl]),
            "wk": arr_w(wk[:, sl]),
            "wv": arr_w(wv[:, sl]),
            "wo": np.ascontiguousarray(wo_t).astype(BF16).reshape(-1),
            "bq": bq[sl].reshape(NT, 128, 1),
        })

    res = run_bass_kernel_spmd(nc, in_maps, core_ids=list(range(8)))
    LAST_RESULTS = res

    correction = (bv @ wo + bo).astype(np.float32)  # probs sum to 1
    out = np.zeros((B, S, D), dtype=np.float32)
    for c in range(8):
        out[c // 4] += res.results[c]["o"].astype(np.float32)
    out += correction[None, None, :]
    return out


# revision 14
# speedup vs baseline: 1.0211x; 1.0211x over previous
"""Self-contained Trainium2 Bass kernel for 16-head cross-attention MHA.

Problem: B=2, SQ=SK=2048, D=1024, H=16, key_size=64 (fp32 in/out).

Sharding (8 cores): data-parallel over batch (2) x tensor-parallel over
head groups (4 heads per core). Each core computes its 4 heads'
Q/K/V projections (column slices of wq/wk/wv), attention, and a partial
output projection (row slice of wo). Host sums the 4 partial outputs per
batch and adds the (bv @ wo + bo) correction (probs sum to 1, so bv
contributes exactly bv @ wo; bk cancels in softmax).

Device pipeline per core (bf16 matmuls, fp32 PSUM accumulation — fp8
variants measured 2-5% output error, over the gate). The kernel is
ScalarE-paced: exp over the 4 x 2048 x 2048 score matrix is ~142us of
ACT time; everything else is scheduled to hide under it:

  1. Prefix: host pre-arranges every tensor into its SBUF layout so
     each DMA is 16-32KB-contiguous per partition (~3.5x the DMA
     throughput of the 512B-descriptor rearranged forms). xeT then xdT
     kt-pair chunks alternate the sync/scalar queues (weights ride
     gpsimd; wo is gated behind the K eviction). ~30 junk matmuls keep
     the PE out of its low p-states while the DMAs stream; K-nt0 and
     Q-nt0 (m-half 0 only) chains pipeline behind the chunk arrivals.
  2. Phases (head-pair, m-half): scores^T tiles ([key_pos, query]
     layout) exp'd by ScalarE on [128,1024] PSUM tiles (scale=1/8
     fused) into bf16 SBUF, 38-buffer pool — deep enough that phase
     i+1's exps never stall on phase i's shifted ctx (the 34-buffer
     version lost ~8us/phase to pool lockstep).
  3. ctx for phase i is PHASE-SHIFTED into phase i+1 (V' ones column
     accumulates the softmax denominator Z); the last phase runs prev
     ctx at 2x front / own ctx at 2x back. Phase-0 dribbles: V' pairs
     (jts 0-7), Q-nt0 m-half1 (jts 4-7), then single-PSUM-bank
     sequential chains for K-nt1 (jts 8-11) and Q-nt1 (jts 12-15).
  4. Normalization reads ctx straight from PSUM (no staging copy):
     Z row partition-broadcast via GpSimd, DVE reciprocal + multiply
     into ctx^T bf16.
  5. Tail: out-proj for the already-normed m-half (mts 0-7, ACT-only
     evictions) runs on PE while DVE/GpSimd norm the last phase's ctx;
     then mts 8-15 with ACT/DVE split evictions. Output in bf16 (host
     sums partials in fp32), halving the closing DMA drain.
"""

import os
import sys

for _p in ("/opt/trn_rl_repo", "/root/.axon_site/_ro/trn_rl_repo"):
    if os.path.isdir(_p) and _p not in sys.path:
        sys.path.insert(0, _p)

import numpy as np
import ml_dtypes

BF16 = ml_dtypes.bfloat16

B = 2
S = 2048          # SQ == SK
D = 1024
H = 16
KEY = 64
HPC = 4           # heads per core
NPC = HPC * KEY   # 256 per-core slice of D
KT = D // 128     # 8 contraction tiles for projections
TP = KT // 2      # 4 kt-pair DMA chunks
NT = NPC // 128   # 2 head-pair tiles
MC = S // 512     # 4 m-chunks of 512
JT = S // 128     # 16 key tiles

_NC = None
LAST_RESULTS = None  # BassKernelResults of the most recent run (for test.py)


def _build_nc():
    import concourse.tile as tile
    from concourse import bacc, mybir

    FP32 = mybir.dt.float32
    BF = mybir.dt.bfloat16
    AF = mybir.ActivationFunctionType

    nc = bacc.Bacc("TRN2", target_bir_lowering=False, debug=False, num_devices=8)

    xdT_d = nc.dram_tensor("xdT", [128 * KT * S], BF, kind="ExternalInput").ap()
    xeT_d = nc.dram_tensor("xeT", [128 * KT * S], BF, kind="ExternalInput").ap()
    wq_d = nc.dram_tensor("wq", [128 * KT * NPC], BF, kind="ExternalInput").ap()
    wk_d = nc.dram_tensor("wk", [128 * KT * NPC], BF, kind="ExternalInput").ap()
    wv_d = nc.dram_tensor("wv", [128 * KT * NPC], BF, kind="ExternalInput").ap()
    wo_d = nc.dram_tensor("wo", [128 * NT * D], BF, kind="ExternalInput").ap()
    bq_d = nc.dram_tensor("bq", [NT, 128, 1], FP32, kind="ExternalInput").ap()
    o_d = nc.dram_tensor("o", [S, D], BF, kind="ExternalOutput").ap()

    with tile.TileContext(nc) as tc:
        with (
            tc.tile_pool(name="consts", bufs=1) as consts,
            tc.tile_pool(name="acts", bufs=1) as acts,
            tc.tile_pool(name="zp", bufs=2) as zp,
            tc.tile_pool(name="zbp", bufs=2) as zbp,
            tc.tile_pool(name="osb", bufs=3) as osb,
        ):
            # Host pre-arranges every tensor into its SBUF layout, so each
            # DMA is one 16-32KB contiguous descriptor per partition (the
            # rearranged 512B-descriptor forms measured ~3x lower DMA
            # throughput). xeT/xdT halves split across the sync/scalar
            # queues so the two queue rails stream in parallel.
            xeT_r = xeT_d.rearrange("(p t m) -> p t m", p=128, t=KT)
            xdT_r = xdT_d.rearrange("(p t m) -> p t m", p=128, t=KT)

            wk_sb = consts.tile([128, KT, NPC], BF, tag="wk")
            nc.gpsimd.dma_start(wk_sb[:], wk_d.rearrange("(p t n) -> p t n", p=128, t=KT))
            wq_sb = consts.tile([128, KT, NPC], BF, tag="wq")
            nc.gpsimd.dma_start(wq_sb[:], wq_d.rearrange("(p t n) -> p t n", p=128, t=KT))
            wv_sb = consts.tile([128, KT, NPC], BF, tag="wv")
            nc.gpsimd.dma_start(wv_sb[:], wv_d.rearrange("(p t n) -> p t n", p=128, t=KT))
            wo_sb = consts.tile([128, NT, D], BF, tag="wo")

            # xeT first (K0 gates on it), then xdT m-half0 (Q0 prefix), then
            # xdT m-half1 (phase-0 dribbles). kt-pair chunks alternate the
            # sync/scalar queues so K0/Q0 chains pipeline behind arrivals.
            xeT_sb = consts.tile([128, KT, S], BF, tag="xeT")
            xdT_sb = consts.tile([128, KT, S], BF, tag="xdT")
            qs = [nc.sync, nc.scalar]
            for tp in range(TP):
                qs[tp % 2].dma_start(xeT_sb[:, 2 * tp:2 * tp + 2, :],
                                     xeT_r[:, 2 * tp:2 * tp + 2, :])
            for mh2 in range(2):
                msl = slice(mh2 * 1024, (mh2 + 1) * 1024)
                for tp in range(TP):
                    qs[tp % 2].dma_start(xdT_sb[:, 2 * tp:2 * tp + 2, msl],
                                         xdT_r[:, 2 * tp:2 * tp + 2, msl])
            bq_sb = consts.tile([128, NT, 1], FP32, tag="bq")
            nc.scalar.dma_start(bq_sb[:], bq_d.rearrange("t p o -> p t o"))

            # ---- activations kept resident ----
            QT_sb = acts.tile([128, NT, S], BF, tag="QT")    # [head_dim, m]
            KT_sb = acts.tile([128, NT, S], BF, tag="KT")    # [head_dim, j]
            v_sb = acts.tile([128, JT, HPC, KEY + 1], BF, tag="v")  # V' + ones col
            ctxT_sb = acts.tile([128, NT, S], BF, tag="ctxT")

            nc.vector.memset(v_sb[:, :, :, KEY:KEY + 1], 1.0)

            # ================= single PSUM pool =================
            # "ss": 2x[128,1024] (4 banks) scores / out-proj
            # "cc": 4x[128,512] (4 banks) proj chains, V chains, ctx chains
            with (
                tc.tile_pool(name="expp", bufs=38) as expp,
                tc.tile_pool(name="ps", bufs=2, space="PSUM") as ps,
                tc.tile_pool(name="cp", bufs=4, space="PSUM") as cp,
            ):
                def proj_pass(w_sb, x_sb, nt, chains, mcs=range(MC)):
                    for kt in range(KT):
                        for i, mc in enumerate(mcs):
                            nc.tensor.matmul(
                                chains[i][:],
                                w_sb[:, kt, nt * 128:(nt + 1) * 128],
                                x_sb[:, kt, mc * 512:(mc + 1) * 512],
                                start=(kt == 0),
                                stop=(kt == KT - 1),
                            )

                def evict_proj(chains, dst, nt, bias, mcs=range(MC)):
                    for i, mc in enumerate(mcs):
                        out_ap = dst[:, nt, mc * 512:(mc + 1) * 512]
                        if bias is not None:
                            nc.vector.tensor_scalar_add(out_ap, chains[i][:],
                                                        bias[:, nt, :])
                        else:
                            nc.vector.tensor_copy(out_ap, chains[i][:])

                # ---- PE warm-up: ~30 junk matmuls on a zero tile keep the
                # Tensor engine out of its low p-states while the prefix
                # DMAs stream; K0 then runs at full clock.
                warm_sb = consts.tile([128, 512], BF, tag="warm")
                nc.vector.memset(warm_sb[:], 0.0)
                wp = cp.tile([128, 512], FP32, tag="cc", name="warm")
                for _ in range(30):
                    nc.tensor.matmul(wp[:], warm_sb[:, 0:128], warm_sb[:],
                                     start=True, stop=True)

                # ---- prefix: K-nt0, then Q-nt0 for m-half 0 only ----
                k0 = [cp.tile([128, 512], FP32, tag="cc", name=f"k0_{i}")
                      for i in range(4)]
                proj_pass(wk_sb, xeT_sb, 0, k0)
                evict_proj(k0, KT_sb, 0, None)
                # wo (tail-only, 512KB) rides the gpsimd queue gated on a
                # dummy read of the K0 eviction so it cannot steal DMA
                # bandwidth from the critical xeT/xdT prefix loads.
                gate = consts.tile([1, 8], BF, tag="gate")
                nc.gpsimd.tensor_copy(gate[:], KT_sb[0:1, 0, 0:8])
                nc.gpsimd.dma_start(wo_sb[:],
                                    wo_d.rearrange("(p t n) -> p t n", p=128, t=NT))
                q0 = [cp.tile([128, 512], FP32, tag="cc", name=f"q0_{i}")
                      for i in range(2)]
                proj_pass(wq_sb, xdT_sb, 0, q0, mcs=(0, 1))
                evict_proj(q0, QT_sb, 0, bq_sb, mcs=(0, 1))

                def emit_outproj(mts, split):
                    for mt in mts:
                        ot = osb.tile([128, D], BF, tag="ot")
                        po = ps.tile([128, 1024], FP32, tag="ss",
                                     name=f"po_{mt}")
                        for dt in range(NT):
                            for ec in range(2):
                                nc.tensor.matmul(
                                    po[:, ec * 512:(ec + 1) * 512],
                                    ctxT_sb[:, dt, mt * 128:(mt + 1) * 128],
                                    wo_sb[:, dt, ec * 512:(ec + 1) * 512],
                                    start=(dt == 0),
                                    stop=(dt == NT - 1),
                                )
                        if split:
                            nc.scalar.copy(ot[:, 0:512], po[:, 0:512])
                            nc.vector.tensor_copy(ot[:, 512:1024],
                                                  po[:, 512:1024])
                        else:
                            nc.scalar.copy(ot[:], po[:])
                        if mt % 2 == 0:
                            nc.sync.dma_start(
                                o_d[mt * 128:(mt + 1) * 128, :], ot[:])
                        else:
                            nc.gpsimd.dma_start(
                                o_d[mt * 128:(mt + 1) * 128, :], ot[:])

                # ---- phases: scores(si) + shifted ctx(si-1) + dribbles ----
                order = [(0, 0), (0, 1), (1, 0), (1, 1)]
                rows = [0, KEY]
                prev = None  # (hp, mh, exp_tiles)
                drib = {}    # state for phase-0 dribbles

                def emit_ctx_half(hp, mh, jt, exp_row, ccs, hh):
                    h = hp * 2 + hh
                    for q in range(2):
                        nc.tensor.matmul(
                            ccs[hh * 2 + q][0:KEY + 1, :],
                            v_sb[:, jt, h, :],
                            exp_row[hh][:, q * 512:(q + 1) * 512],
                            start=(jt == 0),
                            stop=(jt == JT - 1),
                        )

                def emit_ctx_step(hp, mh, jt, exp_row, ccs):
                    for hh in range(2):
                        h = hp * 2 + hh
                        for q in range(2):
                            nc.tensor.matmul(
                                ccs[hh * 2 + q][0:KEY + 1, :],
                                v_sb[:, jt, h, :],
                                exp_row[hh][:, q * 512:(q + 1) * 512],
                                start=(jt == 0),
                                stop=(jt == JT - 1),
                            )

                def emit_norm(hp, mh, ccs, qs=(0, 1)):
                    m0 = mh * 1024
                    for q in qs:
                        for hh in range(2):
                            row = rows[hh]
                            c = ccs[hh * 2 + q]
                            zraw = zp.tile([1, 512], FP32, tag="z")
                            nc.vector.tensor_copy(zraw[:], c[KEY:KEY + 1, :])
                            zb = zbp.tile([KEY, 512], FP32, tag="zb")
                            nc.gpsimd.partition_broadcast(zb[:], zraw[:])
                            zbr = zbp.tile([KEY, 512], FP32, tag="zbr")
                            nc.vector.reciprocal_approx_fast(zbr[:], zb[:])
                            nc.vector.tensor_mul(
                                ctxT_sb[row:row + KEY, hp,
                                        m0 + q * 512:m0 + (q + 1) * 512],
                                c[0:KEY, :],
                                zbr[:],
                            )

                for si, (hp, mh) in enumerate(order):
                    m0 = mh * 1024
                    last = si == len(order) - 1
                    cur_cc = None
                    prev_cc = None
                    if prev is not None:
                        prev_cc = [cp.tile([128, 512], FP32, tag="cc",
                                           name=f"cc_{si}_{i}")
                                   for i in range(4)]
                    cur_exps = []
                    for jt in range(JT):
                        exp_row = []
                        for hh in range(2):
                            row = rows[hh]
                            ss = ps.tile([128, 1024], FP32, tag="ss")
                            for q in range(2):
                                nc.tensor.matmul(
                                    ss[:, q * 512:(q + 1) * 512],
                                    KT_sb[row:row + KEY, hp,
                                          jt * 128:(jt + 1) * 128],
                                    QT_sb[row:row + KEY, hp,
                                          m0 + q * 512:m0 + (q + 1) * 512],
                                    start=True, stop=True,
                                )
                            et = expp.tile([128, 1024], BF, tag="exp")
                            nc.scalar.activation(et[:], ss[:], AF.Exp,
                                                 scale=0.125)
                            exp_row.append(et)
                            # half the shifted-ctx matmuls between the two
                            # score pairs: their streams cover the scores
                            # LDWEIGHTS latency.
                            if prev is not None and not last:
                                emit_ctx_half(prev[0], prev[1], jt,
                                              prev[2][jt], prev_cc, hh)
                        cur_exps.append(exp_row)
                        if last:
                            # front half: finish prev phase's ctx at 2x rate;
                            # back half: this phase's own ctx at 2x rate.
                            if jt < 8:
                                for j2 in (jt * 2, jt * 2 + 1):
                                    emit_ctx_step(prev[0], prev[1], j2,
                                                  prev[2][j2], prev_cc)
                                if jt == 7:
                                    emit_norm(prev[0], prev[1], prev_cc)
                            else:
                                if jt == 8:
                                    cur_cc = [cp.tile([128, 512], FP32,
                                                      tag="cc",
                                                      name=f"cc_last_{i}")
                                              for i in range(4)]
                                for j2 in ((jt - 8) * 2, (jt - 8) * 2 + 1):
                                    emit_ctx_step(hp, mh, j2, cur_exps[j2],
                                                  cur_cc)
                        if si == 0:
                            # jts 0-3: Q-nt0 m-half1 (needed by phase 1);
                            # jts 0-9: V tiles (phase 1's shifted ctx);
                            # jts 10-13: K-nt1; jts 12-15: Q-nt1 (first
                            # needed by phase 2 / phase 3 scores).
                            if 4 <= jt < 8:
                                if jt == 4:
                                    drib["q0b"] = [
                                        cp.tile([128, 512], FP32, tag="cc",
                                                name=f"q0b_{i}")
                                        for i in range(2)]
                                for kk in range(2):
                                    kt = (jt - 4) * 2 + kk
                                    for i, mc in enumerate((2, 3)):
                                        nc.tensor.matmul(
                                            drib["q0b"][i][:],
                                            wq_sb[:, kt, 0:128],
                                            xdT_sb[:, kt,
                                                   mc * 512:(mc + 1) * 512],
                                            start=(kt == 0), stop=(kt == 7),
                                        )
                                if jt == 7:
                                    evict_proj(drib["q0b"], QT_sb, 0, bq_sb,
                                               mcs=(2, 3))
                            # V tile pairs at jts 0-7
                            if jt < 8:
                                for vtile in (2 * jt, 2 * jt + 1):
                                    pv = cp.tile([128, 512], FP32, tag="cc",
                                                 name=f"pv_{vtile}")
                                    for kt in range(KT):
                                        nc.tensor.matmul(
                                            pv[:, 0:NPC],
                                            xeT_sb[:, kt,
                                                   vtile * 128:
                                                   (vtile + 1) * 128],
                                            wv_sb[:, kt, :],
                                            start=(kt == 0),
                                            stop=(kt == KT - 1),
                                        )
                                    nc.vector.tensor_copy(
                                        v_sb[:, vtile, :, 0:KEY],
                                        pv[:, 0:NPC].rearrange(
                                            "p (h k) -> p h k", h=HPC),
                                    )
                            # K-nt1 / Q-nt1: one single-bank chain per jt
                            # (sequential m-chunks keep PSUM to 1 bank each).
                            if 8 <= jt < 12:
                                mc = jt - 8
                                kc = cp.tile([128, 512], FP32, tag="cc",
                                             name=f"k1_{mc}")
                                for kt in range(KT):
                                    nc.tensor.matmul(
                                        kc[:],
                                        wk_sb[:, kt, 128:256],
                                        xeT_sb[:, kt,
                                               mc * 512:(mc + 1) * 512],
                                        start=(kt == 0), stop=(kt == 7),
                                    )
                                evict_proj([kc], KT_sb, 1, None, mcs=(mc,))
                            if 12 <= jt < 16:
                                mc = jt - 12
                                qc = cp.tile([128, 512], FP32, tag="cc",
                                             name=f"q1_{mc}")
                                for kt in range(KT):
                                    nc.tensor.matmul(
                                        qc[:],
                                        wq_sb[:, kt, 128:256],
                                        xdT_sb[:, kt,
                                               mc * 512:(mc + 1) * 512],
                                        start=(kt == 0), stop=(kt == 7),
                                    )
                                evict_proj([qc], QT_sb, 1, bq_sb, mcs=(mc,))
                    if last:
                        # out-proj for the m-half whose ctx is already
                        # normed (mh0, mts 0-7) runs on PE while DVE/GpSimd
                        # norm this phase's ctx quarter by quarter;
                        # evictions stay on ACT so DVE is free for the norm.
                        emit_outproj(range(0, 8), split=False)
                        emit_norm(hp, mh, cur_cc, qs=(0,))
                        emit_outproj(range(8, 12), split=False)
                        emit_norm(hp, mh, cur_cc, qs=(1,))
                        emit_outproj(range(12, 16), split=True)
                    elif prev is not None:
                        emit_norm(prev[0], prev[1], prev_cc)
                    prev = (hp, mh, cur_exps)

    nc.compile()
    return nc


def _get_nc():
    global _NC
    if _NC is None:
        _NC = _build_nc()
    return _NC


def _maybe_register_ntff_hook():
    """Optional: register the axon NTFF profile hook so BASS_TRACE=1 yields
    HW exec times. No-op if unavailable (e.g. the grading environment)."""
    if "antenv.axon_hooks" in sys.modules:
        return
    try:
        import types

        if "/root/.axon_site" not in sys.path and os.path.isdir("/root/.axon_site"):
            sys.path.append("/root/.axon_site")
        from trn_agent_boot.trn_boot import _ntff_profile_via_ctypes

        hook = _ntff_profile_via_ctypes("/opt/axon/libaxon_pjrt.so")
        mod = types.ModuleType("antenv.axon_hooks")
        mod.get_axon_ntff_profile_hook = lambda: hook
        mod.set_axon_ntff_profile_hook = lambda h: None
        sys.modules["antenv.axon_hooks"] = mod
    except Exception:
        pass


def kernel(decoder_output, encoder_output, wq, bq, wk, bk, wv, bv, wo, bo):
    from concourse.bass_utils import run_bass_kernel_spmd

    global LAST_RESULTS

    decoder_output = np.asarray(decoder_output, dtype=np.float32)
    encoder_output = np.asarray(encoder_output, dtype=np.float32)
    wq = np.asarray(wq, dtype=np.float32)
    wk = np.asarray(wk, dtype=np.float32)
    wv = np.asarray(wv, dtype=np.float32)
    wo = np.asarray(wo, dtype=np.float32)
    bq = np.asarray(bq, dtype=np.float32)
    bv = np.asarray(bv, dtype=np.float32)
    bo = np.asarray(bo, dtype=np.float32)
    # bk is softmax-invariant (adds a per-query constant to every logit).

    if os.environ.get("BASS_TRACE"):
        _maybe_register_ntff_hook()

    nc = _get_nc()

    def arr_x(x):  # [S, D] -> flat [p, kt, m] per-partition-contiguous
        t = x.T.reshape(KT, 128, S).transpose(1, 0, 2)  # [p, kt, m]
        return np.ascontiguousarray(t).astype(BF16).reshape(-1)

    def arr_w(w):  # [D, n] -> flat [p, kt, n]
        n = w.shape[1]
        t = w.reshape(KT, 128, n).transpose(1, 0, 2)
        return np.ascontiguousarray(t).astype(BF16).reshape(-1)

    xT = {}
    for b in range(B):
        xT[("d", b)] = arr_x(decoder_output[b])
        xT[("e", b)] = arr_x(encoder_output[b])

    in_maps = []
    for c in range(8):
        b, hg = c // 4, c % 4
        sl = slice(hg * NPC, (hg + 1) * NPC)
        wo_sl = np.ascontiguousarray(wo[sl, :])  # [NPC, D]
        wo_t = wo_sl.reshape(NT, 128, D).transpose(1, 0, 2)
        in_maps.append({
            "xdT": xT[("d", b)],
            "xeT": xT[("e", b)],
            "wq": arr_w(wq[:, sl]),
            "wk": arr_w(wk[:, sl]),
            "wv": arr_w(wv[:, sl]),
            "wo": np.ascontiguousarray(wo_t).astype(BF16).reshape(-1),
            "bq": bq[sl].reshape(NT, 128, 1),
        })

    res = run_bass_kernel_spmd(nc, in_maps, core_ids=list(range(8)))
    LAST_RESULTS = res

    correction = (bv @ wo + bo).astype(np.float32)  # probs sum to 1
    out = np.zeros((B, S, D), dtype=np.float32)
    for c in range(8):
        out[c // 4] += res.results[c]["o"].astype(np.float32)
    out += correction[None, None, :]
    return out
